# revision 10
# baseline (speedup 1.0000x reference)
"""Trainium2 Bass kernel for nn_ArrivalTime (8-core data-parallel).

Math restructure (exact): with T=24 timeslots and one user per batch row,
scores for token n are row b(n) / hour(n) of two tiny tables:
  A[b, (h,t)]   = (user_row_b @ Wq_u.T + bq) . k[h,t]      (8 rows per core)
  C[tau, (h,t)] = (ts_tau @ Wq_t.T) . k[h,t]               (24 rows)
plus -1e30 on masked (token, t) pairs.  Both tables, and the fused output
projection Vu[(h,t), :] = v[h,t] @ Wu_h.T + bu/4, are tiny (O(T*d)) and are
precomputed on the host in f32 (like the host-side one-hot / user-row prep).

Device dataflow, per 512-token supertile, in [96 = H*T, token] layout:
  ps[96, 512] = acm[56, 96].T @ ohm[56, tok]    (one-hot select + mask bias)
  esc = exp(ps)                                  (scalar)
  Z[4, 512]  = indz[96, 4].T @ esc               (per-head softmax sums)
  lnz = ln(Z)                                    (scalar; same ACT table set)
  ps += nindb[4, 96].T @ lnz                     (broadcast -lnZ, PSUM accum)
  att = exp(ps)                                  (scalar; normalized softmax)
  po[128, 1024] = att[:, j*128:..].T @ Vu        (4 token tiles)
  ob = bf16(po)                                  (vector/scalar split), DMA out
The [96, tok] layout removes all per-tile PE transposes; the ln-based
normalization keeps the softmax divide off the (slow) DVE reciprocal and
needs no elementwise multiply.  All activations (Exp/Ln/Copy) live in the
one `natural_log_exp_and_others` ACT table set -> a single table load; the
table-choice hint below steers the compiler to it.  A short bf16 warmup
bridges the PE HAM clock gate (1.2 -> 2.4 GHz) into the real matmul stream;
per-supertile input DMAs let the first score start as early as possible.

Per core: shard the B axis (8 rows -> 4096 tokens).  No collectives.
Output is written bf16 and upcast to f32 on the host.
"""

import numpy as np
import ml_dtypes
from contextlib import ExitStack

import concourse.bass as bass
import concourse.mybir as mybir
import concourse.tile as tile
from concourse import bacc
from concourse.bass_utils import run_bass_kernel_spmd

F32 = mybir.dt.float32
BF16 = mybir.dt.bfloat16
AF = mybir.ActivationFunctionType
ALU = mybir.AluOpType

D_MODEL = 1024
N_HEADS = 4
HEAD_DIM = 256
T = 24
B, S = 64, 512
NCORES = 8
BL = B // NCORES            # 8 batch rows per core
NL = BL * S                 # 4096 tokens per core
P = 128
NT = NL // P                # 32 token tiles
HT = N_HEADS * T            # 96
ST = 512                    # tokens per supertile
NST = NL // ST              # 8
TPS = ST // P               # 4 token tiles per supertile
NSEL = 32 + T               # 56 selector rows (24 tau + 8 batch + 24 mask)
ACT_SET = "natural_log_exp_and_others"
SCALE = 1.0 / np.sqrt(HEAD_DIM)
NEG_BIG = np.float32(-1e30)
N_WARM = 4                  # bf16 dummy matmuls to bridge the input DMA


def _pin_act_table():
    """Steer the compiler's ACT-table choice to the one set that holds every
    function this kernel uses (Exp/Ln/Copy), so it emits a single
    ACT_TABLE_LOAD instead of reloading on each Exp<->Ln switch.  Set names
    and positions are preserved; only the advertised contents change."""
    import concourse.hw_specs as hw_specs
    import concourse.bass_interp as bass_interp
    orig = hw_specs.get_activation_tables

    def pinned(arch):
        t = orig(arch)
        assert ACT_SET in t
        return {k: (v if k == ACT_SET else set()) for k, v in t.items()}

    bacc.get_activation_tables = pinned
    bass_interp.get_activation_tables = pinned


def build():
    _pin_act_table()
    nc = bacc.Bacc("TRN2", target_bir_lowering=False, debug=False)

    acmn = nc.dram_tensor("acmn", [NSEL, 2 * HT], BF16, kind="ExternalInput")
    ohm = nc.dram_tensor("ohm", [NSEL, NL], BF16, kind="ExternalInput")
    big1 = nc.dram_tensor("big1", [HT, D_MODEL + N_HEADS], BF16,
                          kind="ExternalInput")
    out = nc.dram_tensor("out", [P, NT, D_MODEL], BF16, kind="ExternalOutput")

    with tile.TileContext(nc) as tc, ExitStack() as ctx:
        const = ctx.enter_context(tc.tile_pool(name="const", bufs=1))
        sb = ctx.enter_context(tc.tile_pool(name="sb", bufs=1))
        obp = ctx.enter_context(tc.tile_pool(name="obp", bufs=1))

        # fine-grained input DMAs: score(0) needs only acmn + ohm chunk 0
        acmn_sb = const.tile([NSEL, 2 * HT], BF16)
        nc.sync.dma_start(acmn_sb[:], acmn[:])
        ohm_sb = []
        for g in range(NST):
            t_ = const.tile([NSEL, ST], BF16, name=f"ohmc{g}")
            ohm_sb.append(t_)
        nc.sync.dma_start(ohm_sb[0][:], ohm[:, 0:ST])
        nc.sync.dma_start(ohm_sb[1][:], ohm[:, ST:2 * ST])
        big1_sb = const.tile([HT, D_MODEL + N_HEADS], BF16)
        nc.sync.dma_start(big1_sb[:], big1[:])
        for g in range(2, NST):
            nc.sync.dma_start(ohm_sb[g][:], ohm[:, g * ST:(g + 1) * ST])
        acm_sb = acmn_sb[:, 0:HT]
        nindb_sb = acmn_sb[0:4, HT:2 * HT]
        vu_sb = big1_sb[:, 0:D_MODEL]
        indz_sb = big1_sb[:, D_MODEL:D_MODEL + N_HEADS]

        with tc.tile_pool(name="pst", bufs=1, space="PSUM") as pst:
            # PE warm-up + in-loop filler: the HAM clock gate throttles the
            # PE to 1.2 GHz whenever an activity window sees idle time, and
            # only a fully-busy ~3.4us window restores 2.4 GHz.  Dummy
            # matmuls keep the PE stream dense: a long burst up front, then
            # short fillers at the two dependency stalls of each supertile.
            warm_sb = const.tile([P, 512], BF16)
            nc.vector.memset(warm_sb[:], 0.0)
            warm_ps = pst.tile([P, 512], F32, tag="warm", bufs=1)

            def warm(n, cols=512):
                for i in range(n):
                    nc.tensor.matmul(warm_ps[:, 0:cols], warm_sb[:, 0:P],
                                     warm_sb[:, 0:cols], start=True, stop=True)

            warm(N_WARM)
            def emit_score(g):
                ps = pst.tile([HT, ST], F32, tag="ps", name=f"ps{g}", bufs=2)
                nc.tensor.matmul(ps[:], acm_sb, ohm_sb[g][:],
                                 start=True, stop=True)
                return ps

            def emit_exp1(g, ps):
                esc = sb.tile([HT, ST], BF16, tag="esc", name=f"esc{g}", bufs=2)
                nc.scalar.activation(esc[:], ps[:], AF.Exp)
                return esc

            # softmax pipeline stage A for supertile g: score/exp already
            # emitted; run Zsum -> ln -> mm2 -> exp2, producing att[g].
            def emit_zsum(g, esc):
                zps = pst.tile([N_HEADS, ST], F32, tag="z", name=f"z{g}", bufs=1)
                nc.tensor.matmul(zps[:], indz_sb, esc[:], start=True, stop=True)
                return zps

            def emit_ln(g, zps):
                lnz = sb.tile([N_HEADS, ST], BF16, tag="lnz", name=f"lnz{g}",
                              bufs=2)
                nc.scalar.activation(lnz[:], zps[:], AF.Ln)
                return lnz

            def emit_mm2(g, ps, lnz):
                # ps += (-lnZ) broadcast over each head's 24 rows
                nc.tensor.matmul(ps[:], nindb_sb, lnz[:],
                                 start=False, stop=True, skip_group_check=True)

            def emit_exp2(g, ps):
                att = sb.tile([HT, ST], BF16, tag="att", name=f"att{g}", bufs=2)
                nc.scalar.activation(att[:], ps[:], AF.Exp)
                return att

            # prologue: stage A for supertile 0, PE-padded with fillers so
            # the HAM activity window never sees a long idle stretch
            ps = {0: emit_score(0)}
            esc = {0: emit_exp1(0, ps[0])}
            warm(6)
            z0 = emit_zsum(0, esc[0])
            warm(4)
            lnz0 = emit_ln(0, z0)
            warm(4)
            emit_mm2(0, ps[0], lnz0)
            att = {0: emit_exp2(0, ps[0])}
            warm(2)
            ps[1] = emit_score(1)
            esc[1] = emit_exp1(1, ps[1])

            # steady state: stage A of supertile g+1 interleaves between the
            # out-matmuls of supertile g, keeping the PE stream dense
            for g in range(NST):
                if g + 1 < NST:
                    zps = emit_zsum(g + 1, esc[g + 1])
                if g + 2 < NST:
                    ps[g + 2] = emit_score(g + 2)
                ob = obp.tile([P, TPS, D_MODEL], BF16, tag="ob", name=f"ob{g}",
                              bufs=2)
                pos = []

                def outmm(j):
                    po = pst.tile([P, D_MODEL], F32, tag="po",
                                  name=f"po{g}_{j}", bufs=2)
                    for h in range(2):
                        nc.tensor.matmul(po[:, h * 512:(h + 1) * 512],
                                         att[g][:, j * P:(j + 1) * P],
                                         vu_sb[:, h * 512:(h + 1) * 512],
                                         start=True, stop=True)
                    pos.append((j, po))

                outmm(0)
                if g + 1 < NST:
                    lnz = emit_ln(g + 1, zps)
                outmm(1)
                if g + 1 < NST:
                    emit_mm2(g + 1, ps[g + 1], lnz)
                    att[g + 1] = emit_exp2(g + 1, ps[g + 1])
                if g + 2 < NST:
                    esc[g + 2] = emit_exp1(g + 2, ps[g + 2])
                warm(3, 256)
                outmm(2)
                outmm(3)
                warm(1, 256)
                # evacuate PSUM -> SBUF bf16: vector takes j0..j2, scalar j3
                for j, po in pos:
                    dst = ob[:, j, :]
                    if j == 3:
                        nc.scalar.copy(dst, po[:])
                    else:
                        nc.vector.tensor_copy(dst, po[:])
                nc.sync.dma_start(out[:, g * TPS:g * TPS + 2, :], ob[:, 0:2, :])
                nc.sync.dma_start(out[:, g * TPS + 2:g * TPS + 4, :],
                                  ob[:, 2:4, :])

    nc.finalize()
    return nc


def _bf16(x):
    return np.ascontiguousarray(x.astype(ml_dtypes.bfloat16))


def prep_in_maps(inputs):
    ts = np.asarray(inputs["timeslot_embedded"], np.float32)
    user_x1 = np.asarray(inputs["user_x1"]).astype(np.int64)
    hour = np.asarray(inputs["hour_x1"]).astype(np.int64)
    mask = np.asarray(inputs["hour_mask1"]).astype(np.int64)
    up = np.asarray(inputs["up_table"], np.float32)
    Wq = np.asarray(inputs["Wq"], np.float32)
    bq = np.asarray(inputs["bq"], np.float32)
    Wk = np.asarray(inputs["Wk"], np.float32)
    bk = np.asarray(inputs["bk"], np.float32)
    Wv = np.asarray(inputs["Wv"], np.float32)
    bv = np.asarray(inputs["bv"], np.float32)
    Wu = np.asarray(inputs["Wu"], np.float32)
    bu = np.asarray(inputs["bu"], np.float32)

    Wqf = Wq.reshape(N_HEADS * HEAD_DIM, 2 * D_MODEL)
    Wq_u, Wq_t = Wqf[:, :D_MODEL], Wqf[:, D_MODEL:]
    Wkf = Wk.reshape(N_HEADS * HEAD_DIM, D_MODEL)
    Wvf = Wv.reshape(N_HEADS * HEAD_DIM, D_MODEL)

    # tiny tables in f32 on the host (O(T*d) work)
    k = ts @ Wkf.T + bk.ravel()                    # [24, 1024]
    v = ts @ Wvf.T + bv.ravel()                    # [24, 1024]
    Pq = ts @ Wq_t.T                               # [24, 1024]
    user_rows = up[user_x1].copy()
    user_rows[user_x1 == 0] = 0.0                  # padding_idx=0
    uq = user_rows @ Wq_u.T + bq.ravel()           # [64, 1024]

    C = np.empty((T, HT), np.float32)
    A = np.empty((B, HT), np.float32)
    Vu = np.empty((HT, D_MODEL), np.float32)
    for h in range(N_HEADS):
        hs = slice(h * HEAD_DIM, (h + 1) * HEAD_DIM)
        cb = slice(h * T, (h + 1) * T)
        C[:, cb] = Pq[:, hs] @ k[:, hs].T
        A[:, cb] = uq[:, hs] @ k[:, hs].T
        Vu[cb, :] = v[:, hs] @ Wu[:, hs].T
    Vu += bu / N_HEADS                             # heads' attn rows sum to 1

    # mrows[t', h*24+t] = -1e30 if t == t' else 0
    mrows = np.zeros((T, HT), np.float32)
    for h in range(N_HEADS):
        mrows[np.arange(T), h * T + np.arange(T)] = NEG_BIG

    indz = np.zeros((HT, N_HEADS), np.float32)
    for h in range(N_HEADS):
        indz[h * T:(h + 1) * T, h] = 1.0

    big1 = _bf16(np.concatenate([Vu, indz], axis=1))

    nindb = np.zeros((NSEL, HT), np.float32)
    nindb[0:N_HEADS, :] = -indz.T

    tok_b = (np.arange(NL) // S).astype(np.int64)
    in_maps = []
    for c in range(NCORES):
        acm = np.concatenate([C, A[c * BL:(c + 1) * BL], mrows], 0)
        acmn = _bf16(np.concatenate([acm, nindb], axis=1))
        hour_c = hour[c * BL:(c + 1) * BL].reshape(-1)
        ohe = np.zeros((NSEL, NL), np.float32)
        ohe[hour_c, np.arange(NL)] = SCALE             # tau selector -> C rows
        ohe[T + tok_b, np.arange(NL)] = SCALE          # b selector -> A rows
        maskc = mask[c * BL:(c + 1) * BL].reshape(NL, T).astype(np.float32)
        ohe[32:32 + T, :] = maskc.T                    # mask rows -> mrows
        in_maps.append({"acmn": acmn, "ohm": _bf16(ohe), "big1": big1})
    return in_maps


_NC_CACHE = None


def get_nc():
    global _NC_CACHE
    if _NC_CACHE is None:
        _NC_CACHE = build()
    return _NC_CACHE


def run(inputs, trace=False, **kwargs):
    nc = get_nc()
    in_maps = prep_in_maps(inputs)
    res = run_bass_kernel_spmd(nc, in_maps, core_ids=list(range(NCORES)),
                               trace=trace, **kwargs)
    outs = [np.asarray(r["out"]).transpose(1, 0, 2).reshape(NL, D_MODEL)
            for r in res.results]
    full = np.concatenate(outs, 0).reshape(B, S, D_MODEL).astype(np.float32)
    return full, res


def kernel(**inputs):
    full, _ = run(inputs, trace=False)
    return full


# revision 11
# speedup vs baseline: 1.1830x; 1.1830x over previous
"""Trainium2 Bass kernel for nn_ArrivalTime (8-core data-parallel).

Math restructure (exact): with T=24 timeslots and one user per batch row,
scores for token n are row b(n) / hour(n) of two tiny tables:
  A[b, (h,t)]   = (user_row_b @ Wq_u.T + bq) . k[h,t]      (8 rows per core)
  C[tau, (h,t)] = (ts_tau @ Wq_t.T) . k[h,t]               (24 rows)
plus -1e30 on masked (token, t) pairs.  Both tables, and the fused output
projection Vu[(h,t), :] = v[h,t] @ Wu_h.T + bu/4, are tiny (O(T*d)) and are
precomputed on the host in f32 (like the host-side one-hot / user-row prep).

Device dataflow, per 512-token supertile, in [96 = H*T, token] layout:
  ps[96, 512] = acm[56, 96].T @ ohm[56, tok]    (one-hot select + mask bias)
  esc = exp(ps)                                  (scalar)
  Z[4, 512]  = indz[96, 4].T @ esc               (per-head softmax sums)
  lnz = ln(Z)                                    (scalar; same ACT table set)
  ps += nindb[4, 96].T @ lnz                     (broadcast -lnZ, PSUM accum)
  att = exp(ps)                                  (scalar; normalized softmax)
  po[128, 1024] = att[:, j*128:..].T @ Vu        (4 token tiles)
  ob = bf16(po)                                  (vector/scalar split), DMA out
The [96, tok] layout removes all per-tile PE transposes; the ln-based
normalization keeps the softmax divide off the (slow) DVE reciprocal and
needs no elementwise multiply.  All activations (Exp/Ln/Copy) live in the
one `natural_log_exp_and_others` ACT table set -> a single table load; the
table-choice hint below steers the compiler to it.  The PE HAM clock gate
(1.2 -> 2.4 GHz) re-throttles on any ~0.5us PE idle stretch, so dummy
filler matmuls pad the two dependency stalls of each supertile and a
burst up front bridges the input DMA.

Per core: shard the B axis (8 rows -> 4096 tokens).  No collectives.
Output is written bf16 and upcast to f32 on the host.
"""

import numpy as np
import ml_dtypes
from contextlib import ExitStack

import concourse.bass as bass
import concourse.mybir as mybir
import concourse.tile as tile
from concourse import bacc
from concourse.bass_utils import run_bass_kernel_spmd

F32 = mybir.dt.float32
BF16 = mybir.dt.bfloat16
AF = mybir.ActivationFunctionType
ALU = mybir.AluOpType

D_MODEL = 1024
N_HEADS = 4
HEAD_DIM = 256
T = 24
B, S = 64, 512
NCORES = 8
BL = B // NCORES            # 8 batch rows per core
NL = BL * S                 # 4096 tokens per core
P = 128
NT = NL // P                # 32 token tiles
HT = N_HEADS * T            # 96
ST = 512                    # tokens per supertile
NST = NL // ST              # 8
TPS = ST // P               # 4 token tiles per supertile
NSEL = 32 + T               # 56 selector rows (24 tau + 8 batch + 24 mask)
ACT_SET = "natural_log_exp_and_others"
SCALE = 1.0 / np.sqrt(HEAD_DIM)
NEG_BIG = np.float32(-1e30)
N_WARM = 10                 # bf16 dummy matmuls to open the PE HAM clock gate


def _pin_act_table():
    """Steer the compiler's ACT-table choice to the one set that holds every
    function this kernel uses (Exp/Ln/Copy), so it emits a single
    ACT_TABLE_LOAD instead of reloading on each Exp<->Ln switch.  Set names
    and positions are preserved; only the advertised contents change."""
    import concourse.hw_specs as hw_specs
    import concourse.bass_interp as bass_interp
    orig = hw_specs.get_activation_tables

    def pinned(arch):
        t = orig(arch)
        assert ACT_SET in t
        return {k: (v if k == ACT_SET else set()) for k, v in t.items()}

    bacc.get_activation_tables = pinned
    bass_interp.get_activation_tables = pinned


def build():
    _pin_act_table()
    nc = bacc.Bacc("TRN2", target_bir_lowering=False, debug=False)

    acmn = nc.dram_tensor("acmn", [NSEL, 2 * HT], BF16, kind="ExternalInput")
    ohm = nc.dram_tensor("ohm", [NSEL, NL], BF16, kind="ExternalInput")
    big1 = nc.dram_tensor("big1", [HT, D_MODEL + N_HEADS], BF16,
                          kind="ExternalInput")
    out = nc.dram_tensor("out", [P, NT, D_MODEL], BF16, kind="ExternalOutput")

    with tile.TileContext(nc) as tc, ExitStack() as ctx:
        const = ctx.enter_context(tc.tile_pool(name="const", bufs=1))
        sb = ctx.enter_context(tc.tile_pool(name="sb", bufs=1))
        obp = ctx.enter_context(tc.tile_pool(name="obp", bufs=1))

        # fine-grained input DMAs: score(0) needs only acmn + ohm chunk 0
        acmn_sb = const.tile([NSEL, 2 * HT], BF16)
        nc.sync.dma_start(acmn_sb[:], acmn[:])
        ohm_sb = []
        for g in range(NST):
            t_ = const.tile([NSEL, ST], BF16, name=f"ohmc{g}")
            ohm_sb.append(t_)
        nc.sync.dma_start(ohm_sb[0][:], ohm[:, 0:ST])
        nc.sync.dma_start(ohm_sb[1][:], ohm[:, ST:2 * ST])
        big1_sb = const.tile([HT, D_MODEL + N_HEADS], BF16)
        nc.sync.dma_start(big1_sb[:], big1[:])
        for g in range(2, NST):
            nc.sync.dma_start(ohm_sb[g][:], ohm[:, g * ST:(g + 1) * ST])
        acm_sb = acmn_sb[:, 0:HT]
        nindb_sb = acmn_sb[0:4, HT:2 * HT]
        vu_sb = big1_sb[:, 0:D_MODEL]
        indz_sb = big1_sb[:, D_MODEL:D_MODEL + N_HEADS]

        with tc.tile_pool(name="pst", bufs=1, space="PSUM") as pst:
            # PE warm-up + in-loop filler: the HAM clock gate throttles the
            # PE to 1.2 GHz whenever an activity window sees idle time, and
            # only a fully-busy ~3.4us window restores 2.4 GHz.  Dummy
            # matmuls keep the PE stream dense: a long burst up front, then
            # short fillers at the two dependency stalls of each supertile.
            warm_sb = const.tile([P, 512], BF16)
            nc.vector.memset(warm_sb[:], 0.0)
            warm_ps = pst.tile([P, 512], F32, tag="warm", bufs=1)

            def warm(n, cols=512):
                for i in range(n):
                    nc.tensor.matmul(warm_ps[:, 0:cols], warm_sb[:, 0:P],
                                     warm_sb[:, 0:cols], start=True, stop=True)

            warm(N_WARM)

            def emit_score(g):
                ps = pst.tile([HT, ST], F32, tag="ps", name=f"ps{g}", bufs=2)
                nc.tensor.matmul(ps[:], acm_sb, ohm_sb[g][:],
                                 start=True, stop=True)
                return ps

            def emit_exp1(g, ps):
                esc = sb.tile([HT, ST], BF16, tag="esc", name=f"esc{g}", bufs=2)
                nc.scalar.activation(esc[:], ps[:], AF.Exp)
                return esc

            ps = emit_score(0)
            esc = emit_exp1(0, ps)
            warm(4)
            for g in range(NST):
                heavy = g < 2
                zps = pst.tile([N_HEADS, ST], F32, tag="z", name=f"z{g}", bufs=1)
                nc.tensor.matmul(zps[:], indz_sb, esc[:], start=True, stop=True)
                if g + 1 < NST:
                    ps2 = emit_score(g + 1)
                warm(4 if heavy else 2, 256)
                lnz = sb.tile([N_HEADS, ST], BF16, tag="lnz", name=f"lnz{g}",
                              bufs=2)
                nc.scalar.activation(lnz[:], zps[:], AF.Ln)
                if g + 1 < NST:
                    esc = emit_exp1(g + 1, ps2)
                # ps += (-lnZ) broadcast over each head's 24 rows
                nc.tensor.matmul(ps[:], nindb_sb, lnz[:],
                                 start=False, stop=True, skip_group_check=True)
                att = sb.tile([HT, ST], BF16, tag="att", name=f"att{g}", bufs=2)
                nc.scalar.activation(att[:], ps[:], AF.Exp)
                warm(4, 256)

                ob = obp.tile([P, TPS, D_MODEL], BF16, tag="ob", name=f"ob{g}",
                              bufs=2)
                pos = []
                for j in range(TPS):
                    po = pst.tile([P, D_MODEL], F32, tag="po",
                                  name=f"po{g}_{j}", bufs=2)
                    for h in range(2):
                        nc.tensor.matmul(po[:, h * 512:(h + 1) * 512],
                                         att[:, j * P:(j + 1) * P],
                                         vu_sb[:, h * 512:(h + 1) * 512],
                                         start=True, stop=True)
                    pos.append((j, po))
                # evacuate PSUM -> SBUF bf16: scalar takes one tile, vector 3;
                # the final supertile splits 2/2 so the drain tail is shorter
                sj = (0, 2) if g == NST - 1 else (0,)
                for j, po in pos:
                    dst = ob[:, j, :]
                    if j in sj:
                        nc.scalar.copy(dst, po[:])
                    else:
                        nc.vector.tensor_copy(dst, po[:])
                nc.sync.dma_start(out[:, g * TPS:g * TPS + 2, :], ob[:, 0:2, :])
                nc.sync.dma_start(out[:, g * TPS + 2:g * TPS + 4, :],
                                  ob[:, 2:4, :])
                if g + 1 < NST:
                    ps = ps2

    nc.finalize()
    return nc


def _bf16(x):
    return np.ascontiguousarray(x.astype(ml_dtypes.bfloat16))


def prep_in_maps(inputs):
    ts = np.asarray(inputs["timeslot_embedded"], np.float32)
    user_x1 = np.asarray(inputs["user_x1"]).astype(np.int64)
    hour = np.asarray(inputs["hour_x1"]).astype(np.int64)
    mask = np.asarray(inputs["hour_mask1"]).astype(np.int64)
    up = np.asarray(inputs["up_table"], np.float32)
    Wq = np.asarray(inputs["Wq"], np.float32)
    bq = np.asarray(inputs["bq"], np.float32)
    Wk = np.asarray(inputs["Wk"], np.float32)
    bk = np.asarray(inputs["bk"], np.float32)
    Wv = np.asarray(inputs["Wv"], np.float32)
    bv = np.asarray(inputs["bv"], np.float32)
    Wu = np.asarray(inputs["Wu"], np.float32)
    bu = np.asarray(inputs["bu"], np.float32)

    Wqf = Wq.reshape(N_HEADS * HEAD_DIM, 2 * D_MODEL)
    Wq_u, Wq_t = Wqf[:, :D_MODEL], Wqf[:, D_MODEL:]
    Wkf = Wk.reshape(N_HEADS * HEAD_DIM, D_MODEL)
    Wvf = Wv.reshape(N_HEADS * HEAD_DIM, D_MODEL)

    # tiny tables in f32 on the host (O(T*d) work)
    k = ts @ Wkf.T + bk.ravel()                    # [24, 1024]
    v = ts @ Wvf.T + bv.ravel()                    # [24, 1024]
    Pq = ts @ Wq_t.T                               # [24, 1024]
    user_rows = up[user_x1].copy()
    user_rows[user_x1 == 0] = 0.0                  # padding_idx=0
    uq = user_rows @ Wq_u.T + bq.ravel()           # [64, 1024]

    C = np.empty((T, HT), np.float32)
    A = np.empty((B, HT), np.float32)
    Vu = np.empty((HT, D_MODEL), np.float32)
    for h in range(N_HEADS):
        hs = slice(h * HEAD_DIM, (h + 1) * HEAD_DIM)
        cb = slice(h * T, (h + 1) * T)
        C[:, cb] = Pq[:, hs] @ k[:, hs].T
        A[:, cb] = uq[:, hs] @ k[:, hs].T
        Vu[cb, :] = v[:, hs] @ Wu[:, hs].T
    Vu += bu / N_HEADS                             # heads' attn rows sum to 1

    # mrows[t', h*24+t] = -1e30 if t == t' else 0
    mrows = np.zeros((T, HT), np.float32)
    for h in range(N_HEADS):
        mrows[np.arange(T), h * T + np.arange(T)] = NEG_BIG

    indz = np.zeros((HT, N_HEADS), np.float32)
    for h in range(N_HEADS):
        indz[h * T:(h + 1) * T, h] = 1.0

    big1 = _bf16(np.concatenate([Vu, indz], axis=1))

    nindb = np.zeros((NSEL, HT), np.float32)
    nindb[0:N_HEADS, :] = -indz.T

    tok_b = (np.arange(NL) // S).astype(np.int64)
    in_maps = []
    for c in range(NCORES):
        acm = np.concatenate([C, A[c * BL:(c + 1) * BL], mrows], 0)
        acmn = _bf16(np.concatenate([acm, nindb], axis=1))
        hour_c = hour[c * BL:(c + 1) * BL].reshape(-1)
        ohe = np.zeros((NSEL, NL), np.float32)
        ohe[hour_c, np.arange(NL)] = SCALE             # tau selector -> C rows
        ohe[T + tok_b, np.arange(NL)] = SCALE          # b selector -> A rows
        maskc = mask[c * BL:(c + 1) * BL].reshape(NL, T).astype(np.float32)
        ohe[32:32 + T, :] = maskc.T                    # mask rows -> mrows
        in_maps.append({"acmn": acmn, "ohm": _bf16(ohe), "big1": big1})
    return in_maps


_NC_CACHE = None


def get_nc():
    global _NC_CACHE
    if _NC_CACHE is None:
        _NC_CACHE = build()
    return _NC_CACHE


def run(inputs, trace=False, **kwargs):
    nc = get_nc()
    in_maps = prep_in_maps(inputs)
    res = run_bass_kernel_spmd(nc, in_maps, core_ids=list(range(NCORES)),
                               trace=trace, **kwargs)
    outs = [np.asarray(r["out"]).transpose(1, 0, 2).reshape(NL, D_MODEL)
            for r in res.results]
    full = np.concatenate(outs, 0).reshape(B, S, D_MODEL).astype(np.float32)
    return full, res


def kernel(**inputs):
    full, _ = run(inputs, trace=False)
    return full


# revision 12
# speedup vs baseline: 1.2055x; 1.0190x over previous
"""Trainium2 Bass kernel for nn_ArrivalTime (8-core data-parallel).

Math restructure (exact): with T=24 timeslots and one user per batch row,
scores for token n are row b(n) / hour(n) of two tiny tables:
  A[b, (h,t)]   = (user_row_b @ Wq_u.T + bq) . k[h,t]      (8 rows per core)
  C[tau, (h,t)] = (ts_tau @ Wq_t.T) . k[h,t]               (24 rows)
plus -1e30 on masked (token, t) pairs.  Both tables, and the fused output
projection Vu[(h,t), :] = v[h,t] @ Wu_h.T + bu/4, are tiny (O(T*d)) and are
precomputed on the host in f32 (like the host-side one-hot / user-row prep).

Device dataflow, per 512-token supertile, in [96 = H*T, token] layout:
  ps[96, 512] = acm[56, 96].T @ ohm[56, tok]    (one-hot select + mask bias)
  esc = exp(ps)                                  (scalar)
  Z[4, 512]  = indz[96, 4].T @ esc               (per-head softmax sums)
  lnz = ln(Z)                                    (scalar; same ACT table set)
  ps += nindb[4, 96].T @ lnz                     (broadcast -lnZ, PSUM accum)
  att = exp(ps)                                  (scalar; normalized softmax)
  po[128, 1024] = att[:, j*128:..].T @ Vu        (4 token tiles)
  ob = bf16(po)                                  (vector/scalar split), DMA out
The [96, tok] layout removes all per-tile PE transposes; the ln-based
normalization keeps the softmax divide off the (slow) DVE reciprocal and
needs no elementwise multiply.  All activations (Exp/Ln/Copy) live in the
one `natural_log_exp_and_others` ACT table set -> a single table load; the
table-choice hint below steers the compiler to it.  The PE HAM clock gate
(1.2 -> 2.4 GHz) re-throttles on any ~0.5us PE idle stretch, so dummy
filler matmuls pad the two dependency stalls of each supertile and a
burst up front bridges the input DMA.

Per core: shard the B axis (8 rows -> 4096 tokens).  No collectives.
Output is written bf16 and upcast to f32 on the host.
"""

import numpy as np
import ml_dtypes
from contextlib import ExitStack

import concourse.bass as bass
import concourse.mybir as mybir
import concourse.tile as tile
from concourse import bacc
from concourse.bass_utils import run_bass_kernel_spmd

F32 = mybir.dt.float32
BF16 = mybir.dt.bfloat16
AF = mybir.ActivationFunctionType
ALU = mybir.AluOpType

D_MODEL = 1024
N_HEADS = 4
HEAD_DIM = 256
T = 24
B, S = 64, 512
NCORES = 8
BL = B // NCORES            # 8 batch rows per core
NL = BL * S                 # 4096 tokens per core
P = 128
NT = NL // P                # 32 token tiles
HT = N_HEADS * T            # 96
ST = 512                    # tokens per supertile
NST = NL // ST              # 8
TPS = ST // P               # 4 token tiles per supertile
NSEL = 32 + T               # 56 selector rows (24 tau + 8 batch + 24 mask)
ACT_SET = "natural_log_exp_and_others"
SCALE = 1.0 / np.sqrt(HEAD_DIM)
NEG_BIG = np.float32(-1e30)
N_WARM = 10                 # bf16 dummy matmuls to open the PE HAM clock gate


def _pin_act_table():
    """Steer the compiler's ACT-table choice to the one set that holds every
    function this kernel uses (Exp/Ln/Copy), so it emits a single
    ACT_TABLE_LOAD instead of reloading on each Exp<->Ln switch.  Set names
    and positions are preserved; only the advertised contents change."""
    import concourse.hw_specs as hw_specs
    import concourse.bass_interp as bass_interp
    orig = hw_specs.get_activation_tables

    def pinned(arch):
        t = orig(arch)
        assert ACT_SET in t
        return {k: (v if k == ACT_SET else set()) for k, v in t.items()}

    bacc.get_activation_tables = pinned
    bass_interp.get_activation_tables = pinned


def build():
    _pin_act_table()
    nc = bacc.Bacc("TRN2", target_bir_lowering=False, debug=False)

    acmn = nc.dram_tensor("acmn", [NSEL, 2 * HT], BF16, kind="ExternalInput")
    ohm = nc.dram_tensor("ohm", [NSEL, NL], BF16, kind="ExternalInput")
    big1 = nc.dram_tensor("big1", [HT, D_MODEL + N_HEADS], BF16,
                          kind="ExternalInput")
    out = nc.dram_tensor("out", [P, NT, D_MODEL], BF16, kind="ExternalOutput")

    with tile.TileContext(nc) as tc, ExitStack() as ctx:
        const = ctx.enter_context(tc.tile_pool(name="const", bufs=1))
        sb = ctx.enter_context(tc.tile_pool(name="sb", bufs=1))
        obp = ctx.enter_context(tc.tile_pool(name="obp", bufs=1))

        # fine-grained input DMAs: score(0) needs only acmn + ohm chunk 0
        acmn_sb = const.tile([NSEL, 2 * HT], BF16)
        nc.sync.dma_start(acmn_sb[:], acmn[:])
        ohm_sb = []
        for g in range(NST):
            t_ = const.tile([NSEL, ST], BF16, name=f"ohmc{g}")
            ohm_sb.append(t_)
        nc.sync.dma_start(ohm_sb[0][:], ohm[:, 0:ST])
        nc.sync.dma_start(ohm_sb[1][:], ohm[:, ST:2 * ST])
        big1_sb = const.tile([HT, D_MODEL + N_HEADS], BF16)
        nc.sync.dma_start(big1_sb[:], big1[:])
        for g in range(2, NST):
            nc.sync.dma_start(ohm_sb[g][:], ohm[:, g * ST:(g + 1) * ST])
        acm_sb = acmn_sb[:, 0:HT]
        nindb_sb = acmn_sb[0:4, HT:2 * HT]
        vu_sb = big1_sb[:, 0:D_MODEL]
        indz_sb = big1_sb[:, D_MODEL:D_MODEL + N_HEADS]

        with tc.tile_pool(name="pst", bufs=1, space="PSUM") as pst:
            # PE warm-up + in-loop filler: the HAM clock gate throttles the
            # PE to 1.2 GHz whenever an activity window sees idle time, and
            # only a fully-busy ~3.4us window restores 2.4 GHz.  Dummy
            # matmuls keep the PE stream dense: a long burst up front, then
            # short fillers at the two dependency stalls of each supertile.
            warm_sb = const.tile([P, 512], BF16)
            nc.vector.memset(warm_sb[:], 0.0)
            warm_ps = pst.tile([P, 512], F32, tag="warm", bufs=1)

            def warm(n, cols=512):
                for i in range(n):
                    nc.tensor.matmul(warm_ps[:, 0:cols], warm_sb[:, 0:P],
                                     warm_sb[:, 0:cols], start=True, stop=True)

            warm(N_WARM)

            def emit_score(g):
                ps = pst.tile([HT, ST], F32, tag="ps", name=f"ps{g}", bufs=2)
                nc.tensor.matmul(ps[:], acm_sb, ohm_sb[g][:],
                                 start=True, stop=True)
                return ps

            def emit_exp1(g, ps):
                esc = sb.tile([HT, ST], BF16, tag="esc", name=f"esc{g}", bufs=2)
                nc.scalar.activation(esc[:], ps[:], AF.Exp)
                return esc

            ps = emit_score(0)
            esc = emit_exp1(0, ps)
            warm(4)
            for g in range(NST):
                heavy = g < 2
                zps = pst.tile([N_HEADS, ST], F32, tag="z", name=f"z{g}", bufs=1)
                nc.tensor.matmul(zps[:], indz_sb, esc[:], start=True, stop=True)
                if g + 1 < NST:
                    ps2 = emit_score(g + 1)
                warm(3 if heavy else 2)
                lnz = sb.tile([N_HEADS, ST], BF16, tag="lnz", name=f"lnz{g}",
                              bufs=2)
                nc.scalar.activation(lnz[:], zps[:], AF.Ln)
                if g + 1 < NST:
                    esc = emit_exp1(g + 1, ps2)
                # ps += (-lnZ) broadcast over each head's 24 rows
                nc.tensor.matmul(ps[:], nindb_sb, lnz[:],
                                 start=False, stop=True, skip_group_check=True)
                # normalized softmax in two pieces: the first 128 tokens
                # release out-mm j0 early, so the PE flows straight from mm2
                # into output matmuls with no filler-sized stall
                att_a = sb.tile([HT, P], BF16, tag="atta", name=f"atta{g}",
                                bufs=2)
                nc.scalar.activation(att_a[:], ps[:, 0:P], AF.Exp)
                att_b = sb.tile([HT, ST - P], BF16, tag="attb", name=f"attb{g}",
                                bufs=2)
                nc.scalar.activation(att_b[:], ps[:, P:ST], AF.Exp)
                warm(1)

                ob = obp.tile([P, TPS, D_MODEL], BF16, tag="ob", name=f"ob{g}",
                              bufs=2)
                pos = []
                for j in range(TPS):
                    po = pst.tile([P, D_MODEL], F32, tag="po",
                                  name=f"po{g}_{j}", bufs=2)
                    lhsT = (att_a[:] if j == 0
                            else att_b[:, (j - 1) * P:j * P])
                    for h in range(2):
                        nc.tensor.matmul(po[:, h * 512:(h + 1) * 512],
                                         lhsT,
                                         vu_sb[:, h * 512:(h + 1) * 512],
                                         start=True, stop=True)
                    pos.append((j, po))
                # evacuate PSUM -> SBUF bf16: scalar takes one tile, vector 3;
                # the final supertiles split 2/2 so the drain tail is shorter
                sj = (0, 2) if g >= NST - 2 else (0,)
                for j, po in pos:
                    dst = ob[:, j, :]
                    if j in sj:
                        nc.scalar.copy(dst, po[:])
                    else:
                        nc.vector.tensor_copy(dst, po[:])
                if g == NST - 1:
                    for j in range(TPS):
                        nc.sync.dma_start(out[:, g * TPS + j:g * TPS + j + 1, :],
                                          ob[:, j:j + 1, :])
                else:
                    nc.sync.dma_start(out[:, g * TPS:g * TPS + 2, :],
                                      ob[:, 0:2, :])
                    nc.sync.dma_start(out[:, g * TPS + 2:g * TPS + 4, :],
                                      ob[:, 2:4, :])
                if g + 1 < NST:
                    ps = ps2

    nc.finalize()
    return nc


def _bf16(x):
    return np.ascontiguousarray(x.astype(ml_dtypes.bfloat16))


def prep_in_maps(inputs):
    ts = np.asarray(inputs["timeslot_embedded"], np.float32)
    user_x1 = np.asarray(inputs["user_x1"]).astype(np.int64)
    hour = np.asarray(inputs["hour_x1"]).astype(np.int64)
    mask = np.asarray(inputs["hour_mask1"]).astype(np.int64)
    up = np.asarray(inputs["up_table"], np.float32)
    Wq = np.asarray(inputs["Wq"], np.float32)
    bq = np.asarray(inputs["bq"], np.float32)
    Wk = np.asarray(inputs["Wk"], np.float32)
    bk = np.asarray(inputs["bk"], np.float32)
    Wv = np.asarray(inputs["Wv"], np.float32)
    bv = np.asarray(inputs["bv"], np.float32)
    Wu = np.asarray(inputs["Wu"], np.float32)
    bu = np.asarray(inputs["bu"], np.float32)

    Wqf = Wq.reshape(N_HEADS * HEAD_DIM, 2 * D_MODEL)
    Wq_u, Wq_t = Wqf[:, :D_MODEL], Wqf[:, D_MODEL:]
    Wkf = Wk.reshape(N_HEADS * HEAD_DIM, D_MODEL)
    Wvf = Wv.reshape(N_HEADS * HEAD_DIM, D_MODEL)

    # tiny tables in f32 on the host (O(T*d) work)
    k = ts @ Wkf.T + bk.ravel()                    # [24, 1024]
    v = ts @ Wvf.T + bv.ravel()                    # [24, 1024]
    Pq = ts @ Wq_t.T                               # [24, 1024]
    user_rows = up[user_x1].copy()
    user_rows[user_x1 == 0] = 0.0                  # padding_idx=0
    uq = user_rows @ Wq_u.T + bq.ravel()           # [64, 1024]

    C = np.empty((T, HT), np.float32)
    A = np.empty((B, HT), np.float32)
    Vu = np.empty((HT, D_MODEL), np.float32)
    for h in range(N_HEADS):
        hs = slice(h * HEAD_DIM, (h + 1) * HEAD_DIM)
        cb = slice(h * T, (h + 1) * T)
        C[:, cb] = Pq[:, hs] @ k[:, hs].T
        A[:, cb] = uq[:, hs] @ k[:, hs].T
        Vu[cb, :] = v[:, hs] @ Wu[:, hs].T
    Vu += bu / N_HEADS                             # heads' attn rows sum to 1

    # mrows[t', h*24+t] = -1e30 if t == t' else 0
    mrows = np.zeros((T, HT), np.float32)
    for h in range(N_HEADS):
        mrows[np.arange(T), h * T + np.arange(T)] = NEG_BIG

    indz = np.zeros((HT, N_HEADS), np.float32)
    for h in range(N_HEADS):
        indz[h * T:(h + 1) * T, h] = 1.0

    big1 = _bf16(np.concatenate([Vu, indz], axis=1))

    nindb = np.zeros((NSEL, HT), np.float32)
    nindb[0:N_HEADS, :] = -indz.T

    tok_b = (np.arange(NL) // S).astype(np.int64)
    in_maps = []
    for c in range(NCORES):
        acm = np.concatenate([C, A[c * BL:(c + 1) * BL], mrows], 0)
        acmn = _bf16(np.concatenate([acm, nindb], axis=1))
        hour_c = hour[c * BL:(c + 1) * BL].reshape(-1)
        ohe = np.zeros((NSEL, NL), np.float32)
        ohe[hour_c, np.arange(NL)] = SCALE             # tau selector -> C rows
        ohe[T + tok_b, np.arange(NL)] = SCALE          # b selector -> A rows
        maskc = mask[c * BL:(c + 1) * BL].reshape(NL, T).astype(np.float32)
        ohe[32:32 + T, :] = maskc.T                    # mask rows -> mrows
        in_maps.append({"acmn": acmn, "ohm": _bf16(ohe), "big1": big1})
    return in_maps


_NC_CACHE = None


def get_nc():
    global _NC_CACHE
    if _NC_CACHE is None:
        _NC_CACHE = build()
    return _NC_CACHE


def run(inputs, trace=False, **kwargs):
    nc = get_nc()
    in_maps = prep_in_maps(inputs)
    res = run_bass_kernel_spmd(nc, in_maps, core_ids=list(range(NCORES)),
                               trace=trace, **kwargs)
    outs = [np.asarray(r["out"]).transpose(1, 0, 2).reshape(NL, D_MODEL)
            for r in res.results]
    full = np.concatenate(outs, 0).reshape(B, S, D_MODEL).astype(np.float32)
    return full, res


def kernel(**inputs):
    full, _ = run(inputs, trace=False)
    return full


# revision 13
# speedup vs baseline: 1.2215x; 1.0132x over previous
"""Trainium2 Bass kernel for nn_ArrivalTime (8-core data-parallel).

Math restructure (exact): with T=24 timeslots and one user per batch row,
scores for token n are row b(n) / hour(n) of two tiny tables:
  A[b, (h,t)]   = (user_row_b @ Wq_u.T + bq) . k[h,t]      (8 rows per core)
  C[tau, (h,t)] = (ts_tau @ Wq_t.T) . k[h,t]               (24 rows)
plus -1e30 on masked (token, t) pairs.  Both tables, and the fused output
projection Vu[(h,t), :] = v[h,t] @ Wu_h.T + bu/4, are tiny (O(T*d)) and are
precomputed on the host in f32 (like the host-side one-hot / user-row prep).

Device dataflow, per 512-token supertile, in [96 = H*T, token] layout:
  ps[96, 512] = acm[56, 96].T @ ohm[56, tok]    (one-hot select + mask bias)
  esc = exp(ps)                                  (scalar)
  Z[4, 512]  = indz[96, 4].T @ esc               (per-head softmax sums)
  lnz = ln(Z)                                    (scalar; same ACT table set)
  ps += nindb[4, 96].T @ lnz                     (broadcast -lnZ, PSUM accum)
  att = exp(ps)                                  (scalar; normalized softmax)
  po[128, 1024] = att[:, j*128:..].T @ Vu        (4 token tiles)
  ob = bf16(po)                                  (vector/scalar split), DMA out
The [96, tok] layout removes all per-tile PE transposes; the ln-based
normalization keeps the softmax divide off the (slow) DVE reciprocal and
needs no elementwise multiply.  All activations (Exp/Ln/Copy) live in the
one `natural_log_exp_and_others` ACT table set -> a single table load; the
table-choice hint below steers the compiler to it.  The PE HAM clock gate
(1.2 -> 2.4 GHz) re-throttles on any ~0.5us PE idle stretch, so dummy
filler matmuls pad the two dependency stalls of each supertile and a
burst up front bridges the input DMA.

Per core: shard the B axis (8 rows -> 4096 tokens).  No collectives.
Output is written bf16 and upcast to f32 on the host.
"""

import numpy as np
import ml_dtypes
from contextlib import ExitStack

import concourse.bass as bass
import concourse.mybir as mybir
import concourse.tile as tile
from concourse import bacc
from concourse.bass_utils import run_bass_kernel_spmd

F32 = mybir.dt.float32
BF16 = mybir.dt.bfloat16
AF = mybir.ActivationFunctionType
ALU = mybir.AluOpType

D_MODEL = 1024
N_HEADS = 4
HEAD_DIM = 256
T = 24
B, S = 64, 512
NCORES = 8
BL = B // NCORES            # 8 batch rows per core
NL = BL * S                 # 4096 tokens per core
P = 128
NT = NL // P                # 32 token tiles
HT = N_HEADS * T            # 96
ST = 512                    # tokens per supertile
NST = NL // ST              # 8
TPS = ST // P               # 4 token tiles per supertile
NSEL = 32 + T               # 56 selector rows (24 tau + 8 batch + 24 mask)
ACT_SET = "natural_log_exp_and_others"
SCALE = 1.0 / np.sqrt(HEAD_DIM)
NEG_BIG = np.float32(-1e30)
N_WARM = 10                 # bf16 dummy matmuls to open the PE HAM clock gate


def _pin_act_table():
    """Steer the compiler's ACT-table choice to the one set that holds every
    function this kernel uses (Exp/Ln/Copy), so it emits a single
    ACT_TABLE_LOAD instead of reloading on each Exp<->Ln switch.  Set names
    and positions are preserved; only the advertised contents change."""
    import concourse.hw_specs as hw_specs
    import concourse.bass_interp as bass_interp
    orig = hw_specs.get_activation_tables

    def pinned(arch):
        t = orig(arch)
        assert ACT_SET in t
        return {k: (v if k == ACT_SET else set()) for k, v in t.items()}

    bacc.get_activation_tables = pinned
    bass_interp.get_activation_tables = pinned


def build():
    _pin_act_table()
    nc = bacc.Bacc("TRN2", target_bir_lowering=False, debug=False)

    acmn = nc.dram_tensor("acmn", [NSEL, 2 * HT], BF16, kind="ExternalInput")
    ohm = nc.dram_tensor("ohm", [NSEL, NL], BF16, kind="ExternalInput")
    big1 = nc.dram_tensor("big1", [HT, D_MODEL + N_HEADS], BF16,
                          kind="ExternalInput")
    out = nc.dram_tensor("out", [P, NT, D_MODEL], BF16, kind="ExternalOutput")

    with tile.TileContext(nc) as tc, ExitStack() as ctx:
        const = ctx.enter_context(tc.tile_pool(name="const", bufs=1))
        sb = ctx.enter_context(tc.tile_pool(name="sb", bufs=1))
        obp = ctx.enter_context(tc.tile_pool(name="obp", bufs=1))

        # fine-grained input DMAs: score(0) needs only acmn + ohm chunk 0
        acmn_sb = const.tile([NSEL, 2 * HT], BF16)
        nc.sync.dma_start(acmn_sb[:], acmn[:])
        ohm_sb = []
        for g in range(NST):
            t_ = const.tile([NSEL, ST], BF16, name=f"ohmc{g}")
            ohm_sb.append(t_)
        nc.sync.dma_start(ohm_sb[0][:], ohm[:, 0:ST])
        nc.sync.dma_start(ohm_sb[1][:], ohm[:, ST:2 * ST])
        big1_sb = const.tile([HT, D_MODEL + N_HEADS], BF16)
        nc.sync.dma_start(big1_sb[:], big1[:])
        for g in range(2, NST):
            nc.sync.dma_start(ohm_sb[g][:], ohm[:, g * ST:(g + 1) * ST])
        acm_sb = acmn_sb[:, 0:HT]
        nindb_sb = acmn_sb[0:4, HT:2 * HT]
        vu_sb = big1_sb[:, 0:D_MODEL]
        indz_sb = big1_sb[:, D_MODEL:D_MODEL + N_HEADS]

        with tc.tile_pool(name="pst", bufs=1, space="PSUM") as pst:
            # PE warm-up + in-loop filler: the HAM clock gate throttles the
            # PE to 1.2 GHz whenever an activity window sees idle time, and
            # only a fully-busy ~3.4us window restores 2.4 GHz.  Dummy
            # matmuls keep the PE stream dense: a long burst up front, then
            # short fillers at the two dependency stalls of each supertile.
            warm_sb = const.tile([P, 512], BF16)
            nc.vector.memset(warm_sb[:], 0.0)
            warm_ps = pst.tile([P, 512], F32, tag="warm", bufs=1)

            def warm(n, cols=512):
                for i in range(n):
                    nc.tensor.matmul(warm_ps[:, 0:cols], warm_sb[:, 0:P],
                                     warm_sb[:, 0:cols], start=True, stop=True)

            warm(N_WARM)

            def emit_score(g):
                ps = pst.tile([HT, ST], F32, tag="ps", name=f"ps{g}", bufs=2)
                nc.tensor.matmul(ps[:], acm_sb, ohm_sb[g][:],
                                 start=True, stop=True)
                return ps

            def emit_exp1(g, ps):
                esc = sb.tile([HT, ST], BF16, tag="esc", name=f"esc{g}", bufs=2)
                nc.scalar.activation(esc[:], ps[:], AF.Exp)
                return esc

            ps = emit_score(0)
            esc = emit_exp1(0, ps)
            warm(4)
            for g in range(NST):
                heavy = g < 2
                zps = pst.tile([N_HEADS, ST], F32, tag="z", name=f"z{g}", bufs=1)
                nc.tensor.matmul(zps[:], indz_sb, esc[:], start=True, stop=True)
                if g + 1 < NST:
                    ps2 = emit_score(g + 1)
                warm(3 if heavy else 2)
                lnz = sb.tile([N_HEADS, ST], BF16, tag="lnz", name=f"lnz{g}",
                              bufs=2)
                nc.scalar.activation(lnz[:], zps[:], AF.Ln)
                if g + 1 < NST:
                    esc = emit_exp1(g + 1, ps2)
                # ps += (-lnZ) broadcast over each head's 24 rows
                nc.tensor.matmul(ps[:], nindb_sb, lnz[:],
                                 start=False, stop=True, skip_group_check=True)
                # normalized softmax in two pieces: the first 128 tokens
                # release out-mm j0 early, so the PE flows straight from mm2
                # into output matmuls with no filler-sized stall
                att_a = sb.tile([HT, P], BF16, tag="atta", name=f"atta{g}",
                                bufs=2)
                nc.scalar.activation(att_a[:], ps[:, 0:P], AF.Exp)
                att_b = sb.tile([HT, ST - P], BF16, tag="attb", name=f"attb{g}",
                                bufs=2)
                nc.scalar.activation(att_b[:], ps[:, P:ST], AF.Exp)
                warm(1)

                ob = obp.tile([P, TPS, D_MODEL], BF16, tag="ob", name=f"ob{g}",
                              bufs=2)
                pos = []
                for j in range(TPS):
                    po = pst.tile([P, D_MODEL], F32, tag="po",
                                  name=f"po{g}_{j}", bufs=2)
                    lhsT = (att_a[:] if j == 0
                            else att_b[:, (j - 1) * P:j * P])
                    for h in range(2):
                        nc.tensor.matmul(po[:, h * 512:(h + 1) * 512],
                                         lhsT,
                                         vu_sb[:, h * 512:(h + 1) * 512],
                                         start=True, stop=True)
                    pos.append((j, po))
                # evacuate PSUM -> SBUF bf16: scalar takes one tile, vector 3;
                # the final supertiles split 2/2 so the drain tail is shorter
                sj = (0, 2) if g >= NST - 2 else (0,)
                for j, po in pos:
                    dst = ob[:, j, :]
                    if j in sj:
                        nc.scalar.copy(dst, po[:])
                    else:
                        nc.vector.tensor_copy(dst, po[:])
                if g == NST - 1:
                    for j in range(TPS):
                        nc.sync.dma_start(out[:, g * TPS + j:g * TPS + j + 1, :],
                                          ob[:, j:j + 1, :])
                else:
                    nc.sync.dma_start(out[:, g * TPS:g * TPS + 2, :],
                                      ob[:, 0:2, :])
                    nc.sync.dma_start(out[:, g * TPS + 2:g * TPS + 4, :],
                                      ob[:, 2:4, :])
                if g + 1 < NST:
                    ps = ps2
                if g == 3:
                    # deliberate re-warm burst: one fully busy ~3.5us HAM
                    # window restores the 2.4 GHz PE clock for the back half
                    warm(9)

    nc.finalize()
    return nc


def _bf16(x):
    return np.ascontiguousarray(x.astype(ml_dtypes.bfloat16))


def prep_in_maps(inputs):
    ts = np.asarray(inputs["timeslot_embedded"], np.float32)
    user_x1 = np.asarray(inputs["user_x1"]).astype(np.int64)
    hour = np.asarray(inputs["hour_x1"]).astype(np.int64)
    mask = np.asarray(inputs["hour_mask1"]).astype(np.int64)
    up = np.asarray(inputs["up_table"], np.float32)
    Wq = np.asarray(inputs["Wq"], np.float32)
    bq = np.asarray(inputs["bq"], np.float32)
    Wk = np.asarray(inputs["Wk"], np.float32)
    bk = np.asarray(inputs["bk"], np.float32)
    Wv = np.asarray(inputs["Wv"], np.float32)
    bv = np.asarray(inputs["bv"], np.float32)
    Wu = np.asarray(inputs["Wu"], np.float32)
    bu = np.asarray(inputs["bu"], np.float32)

    Wqf = Wq.reshape(N_HEADS * HEAD_DIM, 2 * D_MODEL)
    Wq_u, Wq_t = Wqf[:, :D_MODEL], Wqf[:, D_MODEL:]
    Wkf = Wk.reshape(N_HEADS * HEAD_DIM, D_MODEL)
    Wvf = Wv.reshape(N_HEADS * HEAD_DIM, D_MODEL)

    # tiny tables in f32 on the host (O(T*d) work)
    k = ts @ Wkf.T + bk.ravel()                    # [24, 1024]
    v = ts @ Wvf.T + bv.ravel()                    # [24, 1024]
    Pq = ts @ Wq_t.T                               # [24, 1024]
    user_rows = up[user_x1].copy()
    user_rows[user_x1 == 0] = 0.0                  # padding_idx=0
    uq = user_rows @ Wq_u.T + bq.ravel()           # [64, 1024]

    C = np.empty((T, HT), np.float32)
    A = np.empty((B, HT), np.float32)
    Vu = np.empty((HT, D_MODEL), np.float32)
    for h in range(N_HEADS):
        hs = slice(h * HEAD_DIM, (h + 1) * HEAD_DIM)
        cb = slice(h * T, (h + 1) * T)
        C[:, cb] = Pq[:, hs] @ k[:, hs].T
        A[:, cb] = uq[:, hs] @ k[:, hs].T
        Vu[cb, :] = v[:, hs] @ Wu[:, hs].T
    Vu += bu / N_HEADS                             # heads' attn rows sum to 1

    # mrows[t', h*24+t] = -1e30 if t == t' else 0
    mrows = np.zeros((T, HT), np.float32)
    for h in range(N_HEADS):
        mrows[np.arange(T), h * T + np.arange(T)] = NEG_BIG

    indz = np.zeros((HT, N_HEADS), np.float32)
    for h in range(N_HEADS):
        indz[h * T:(h + 1) * T, h] = 1.0

    big1 = _bf16(np.concatenate([Vu, indz], axis=1))

    nindb = np.zeros((NSEL, HT), np.float32)
    nindb[0:N_HEADS, :] = -indz.T

    tok_b = (np.arange(NL) // S).astype(np.int64)
    in_maps = []
    for c in range(NCORES):
        acm = np.concatenate([C, A[c * BL:(c + 1) * BL], mrows], 0)
        acmn = _bf16(np.concatenate([acm, nindb], axis=1))
        hour_c = hour[c * BL:(c + 1) * BL].reshape(-1)
        ohe = np.zeros((NSEL, NL), np.float32)
        ohe[hour_c, np.arange(NL)] = SCALE             # tau selector -> C rows
        ohe[T + tok_b, np.arange(NL)] = SCALE          # b selector -> A rows
        maskc = mask[c * BL:(c + 1) * BL].reshape(NL, T).astype(np.float32)
        ohe[32:32 + T, :] = maskc.T                    # mask rows -> mrows
        in_maps.append({"acmn": acmn, "ohm": _bf16(ohe), "big1": big1})
    return in_maps


_NC_CACHE = None


def get_nc():
    global _NC_CACHE
    if _NC_CACHE is None:
        _NC_CACHE = build()
    return _NC_CACHE


def run(inputs, trace=False, **kwargs):
    nc = get_nc()
    in_maps = prep_in_maps(inputs)
    res = run_bass_kernel_spmd(nc, in_maps, core_ids=list(range(NCORES)),
                               trace=trace, **kwargs)
    outs = [np.asarray(r["out"]).transpose(1, 0, 2).reshape(NL, D_MODEL)
            for r in res.results]
    full = np.concatenate(outs, 0).reshape(B, S, D_MODEL).astype(np.float32)
    return full, res


def kernel(**inputs):
    full, _ = run(inputs, trace=False)
    return full
